# revision 1
# baseline (speedup 1.0000x reference)
"""Trainium2 Bass kernel for a ViT-style transformer block (dense_transformer).

Reference computation (per token row x[t, :1024]):
    h  = LN(x; g1, b1)                       # g1,b1 folded into weights host-side
    q,k,v = h @ Wq.T, h @ Wk.T, h @ Wv.T     # 16 heads x 64
    att   = softmax(q k^T / 8) v  (per batch item of 577 tokens)
    x1 = x + att @ Wp.T + bp
    m  = fast_gelu(LN(x1; g2, b2) @ W1.T + b1) @ W2.T + b2
    out = x1 + m

Sharding: pure data-parallel over batch. B=16 -> 2 batch items (1154 tokens)
per NeuronCore, weights replicated, no collectives.

Device-side layout strategy (per core):
  - Trunk (x, x1) is token-major [tok, C]; LayerNorm via bn_stats.
  - Activations are transposed tile-wise on the PE (128x128) into
    feature-major [C, tok] so they feed matmuls as lhsT/rhs directly.
  - Attention computes S^T = K Q^T with keys on partitions; softmax is a
    bare exp (no max subtraction: |logits| < 9) + a ones-column appended
    to V, so the denominator falls out of the same AV accumulation (row 64
    of the psum).  The PSUM accumulator is evicted unnormalized by the DVE
    (frees the bank fast, keeps the PE streaming); the reciprocal-broadcast-
    multiply tail runs on GpSimd so it never blocks the DVE FIFO.
  - All matmuls in bf16 (fp32 accumulate in PSUM); end-to-end relative
    error vs the fp32 reference ~3e-3.
"""

import numpy as np
import ml_dtypes

import concourse.bass as bass
import concourse.mybir as mybir
import concourse.tile as tile
from concourse import bacc
from concourse.bass_utils import run_bass_kernel_spmd
from concourse.masks import make_identity

F32 = mybir.dt.float32
BF16 = mybir.dt.bfloat16
AF = mybir.ActivationFunctionType
ALU = mybir.AluOpType

C = 1024
H = 16
HD = 64
HID = 4096
B = 16
N = 577
NCORES = 8
BPC = B // NCORES          # batch items per core = 2
T = BPC * N                # tokens per core = 1154
EPS = 1e-5
KC = C // 128              # 8 contraction chunks over C
MC = HID // 128            # 32 chunks over hidden

# per-batch token tiles: (offset, size, batch, idx) within the core's tokens
TOK_TILES = []
for b in range(BPC):
    for i in range(5):
        off = b * N + i * 128
        TOK_TILES.append((off, min(128, N - i * 128), b, i))

# free-dim chunks over the full 1154 tokens (for qk / fc1 rhs streaming)
TCHUNKS = [(0, 512), (512, 512), (1024, T - 1024)]
# free-dim chunks over one batch item's 577 queries
QCHUNKS = [(0, 512), (512, N - 512)]


def _bf(a):
    return np.ascontiguousarray(a.astype(ml_dtypes.bfloat16))


def _block_lhs(wt, n_m):
    """Blocked layout for M=c_out-orientation lhsT weights.

    wt: [c_in, c_out] (already transposed).  Returns [n_m, 128, c_in] where
    block[m][p][kc*128+j] = wt[kc*128+p, m*128+j]; the SBUF tile for
    output-chunk m is [128, c_in] and lhsT for contraction chunk kc is
    tile[:, kc*128:(kc+1)*128].
    """
    c_in = wt.shape[0]
    kc = c_in // 128
    return np.ascontiguousarray(
        wt.reshape(kc, 128, n_m, 128).transpose(2, 1, 0, 3).reshape(n_m, 128, c_in)
    )


def _build(use_bias):
    """Emit the single-core program (SPMD: all 8 cores run it)."""
    from contextlib import ExitStack

    nc = bacc.Bacc(None, target_bir_lowering=False, debug=False)

    x_d = nc.dram_tensor("x", [T, C], F32, kind="ExternalInput")
    wq_d = nc.dram_tensor("wq", [KC, 128, C], BF16, kind="ExternalInput")
    wk_d = nc.dram_tensor("wk", [KC, 128, C], BF16, kind="ExternalInput")
    wv_d = nc.dram_tensor("wv", [KC, 128, C], BF16, kind="ExternalInput")
    wp_d = nc.dram_tensor("wp", [KC, 128, C], BF16, kind="ExternalInput")
    w1_d = nc.dram_tensor("w1", [MC, 128, C], BF16, kind="ExternalInput")
    w2_d = nc.dram_tensor("w2", [MC, 128, C], BF16, kind="ExternalInput")
    bias_d = {}
    for nm, dim in (("bq", C), ("bk", C), ("bv", C), ("bp", C), ("b1", HID), ("b2", C)):
        if use_bias[nm]:
            bias_d[nm] = nc.dram_tensor(nm, [1, dim], BF16, kind="ExternalInput")
    out_d = nc.dram_tensor("out", [T, C], F32, kind="ExternalOutput")

    with tile.TileContext(nc) as tc, ExitStack() as top:
        const = top.enter_context(tc.tile_pool(name="const", bufs=1))
        ps = top.enter_context(tc.tile_pool(name="ps", bufs=1, space="PSUM"))
        p_out1 = top.enter_context(tc.tile_pool(name="p_out1", bufs=1))
        p_h2T = top.enter_context(tc.tile_pool(name="p_h2T", bufs=1))

        ident = const.tile([128, 128], BF16)
        make_identity(nc, ident)
        ones_r = const.tile([1, 512], BF16)
        nc.vector.memset(ones_r, 1.0)
        ones_m = const.tile([1, 128], BF16)
        nc.vector.memset(ones_m, 1.0)
        eps_t = const.tile([128, 1], F32)
        nc.vector.memset(eps_t, EPS)
        bias_sb = {}
        for nm, t in bias_d.items():
            bt = const.tile([1, t.shape[1]], BF16, name=f"b_{nm}")
            nc.sync.dma_start(out=bt, in_=t[:])
            bias_sb[nm] = bt

        out1 = p_out1.tile([128, len(TOK_TILES), C], BF16)
        h2T = p_h2T.tile([128, KC, T], BF16, name="h2T")

        # first use of the gpsimd/DVE custom-op libraries costs a ~7us
        # library load; pay it here, overlapped with the x DMAs
        wu_a = const.tile([1, 8], F32, name="wu_a")
        nc.vector.memset(wu_a, 1.0)
        wu_b = const.tile([8, 8], F32, name="wu_b")
        nc.vector.reciprocal_approx_fast(out=wu_a, in_=wu_a)
        nc.gpsimd.partition_broadcast(wu_b, wu_a)

        def psum_mm(tag="mm"):
            return ps.tile([128, 512], F32, tag=tag, name=tag, bufs=2)

        def ln_tile(pool, x_t, sz, lab):
            """Standardize one token tile -> bf16 h tile (g/b are folded into
            the downstream weights host-side)."""
            st = pool.tile([128, 2, 6], F32, name=f"st{lab}")
            for c2 in range(2):
                nc.vector.bn_stats(
                    out=st[:sz, c2, :], in_=x_t[:sz, c2 * 512 : (c2 + 1) * 512]
                )
            mv = pool.tile([128, 2], F32, name=f"mv{lab}")
            nc.vector.bn_aggr(out=mv[:sz], in_=st[:sz])
            sd = pool.tile([128, 1], F32, name=f"sd{lab}")
            nc.scalar.activation(
                out=sd[:sz], in_=mv[:sz, 1:2], func=AF.Sqrt, bias=eps_t[:sz]
            )
            rstd = pool.tile([128, 1], F32, name=f"rs{lab}")
            nc.vector.reciprocal(out=rstd[:sz], in_=sd[:sz])
            h_t = pool.tile([128, C], BF16, name=f"h{lab}")
            nc.vector.tensor_scalar(
                out=h_t[:sz],
                in0=x_t[:sz],
                scalar1=mv[:sz, 0:1],
                scalar2=rstd[:sz],
                op0=ALU.subtract,
                op1=ALU.mult,
            )
            return h_t

        def transpose_into(h_t, sz, dstT, off, tag="mm"):
            for kc in range(KC):
                ptr = ps.tile([128, 512], BF16, tag=tag, name="tr", bufs=2)
                nc.tensor.transpose(
                    ptr[:, :sz], h_t[:sz, kc * 128 : (kc + 1) * 128], ident[:sz, :sz]
                )
                nc.scalar.copy(out=dstT[:, kc, off : off + sz], in_=ptr[:, :sz])

        # ---------------- phase 1: attention ----------------
        with ExitStack() as ph1:
            p_wres = ph1.enter_context(tc.tile_pool(name="p_wres", bufs=1))
            p_oT = ph1.enter_context(tc.tile_pool(name="p_oT", bufs=1))
            p_dead = ph1.enter_context(tc.tile_pool(name="p_dead", bufs=1))
            p_e = ph1.enter_context(tc.tile_pool(name="p_e", bufs=1))

            wv_sb = p_wres.tile([128, KC, C], BF16, name="wv_sb")
            wp_sb = p_wres.tile([128, KC, C], BF16, name="wp_sb")
            for kc in range(KC):
                nc.sync.dma_start(out=wv_sb[:, kc, :], in_=wv_d[kc])
                nc.sync.dma_start(out=wp_sb[:, kc, :], in_=wp_d[kc])
            oTb = [p_oT.tile([128, KC, N], BF16, name=f"oT{b}")
                   for b in range(BPC)]
            qT = p_dead.tile([128, KC, T], BF16, name="qT")
            kT = p_dead.tile([128, KC, T], BF16, name="kT")
            v_att = p_dead.tile([128, len(TOK_TILES), H * 65], BF16, name="v_att")

            with ExitStack() as ph1a:
                p_x = ph1a.enter_context(tc.tile_pool(name="p_x", bufs=3))
                p_ln = ph1a.enter_context(tc.tile_pool(name="p_ln", bufs=2))
                p_w = ph1a.enter_context(tc.tile_pool(name="p_w", bufs=3))
                p_hT = ph1a.enter_context(tc.tile_pool(name="p_hT", bufs=1))
                hT = p_hT.tile([128, KC, T], BF16, name="hT")

                # LN1 + transpose to feature-major
                for ti, (off, sz, _, _) in enumerate(TOK_TILES):
                    x_t = p_x.tile([128, C], F32, name="x_t")
                    nc.sync.dma_start(out=x_t[:sz], in_=x_d[off : off + sz, :])
                    h_t = ln_tile(p_ln, x_t, sz, "1")
                    transpose_into(h_t, sz, hT, off)

                # v first (its DVE evictions must complete before attention)
                nc.vector.memset(
                    v_att.rearrange("p t (h e) -> p t h e", e=65)[:, :, :, 64:65],
                    1.0,
                )
                for ti, (off, sz, _, _) in enumerate(TOK_TILES):
                    for ci in range(2):
                        pm = psum_mm()
                        for kc in range(KC):
                            nc.tensor.matmul(
                                pm[:sz],
                                hT[:, kc, off : off + sz],
                                wv_sb[:, kc, ci * 512 : (ci + 1) * 512],
                                start=(kc == 0),
                                stop=(kc == KC - 1 and "bv" not in bias_sb),
                            )
                        if "bv" in bias_sb:
                            nc.tensor.matmul(
                                pm[:sz],
                                ones_m[0:1, :sz],
                                bias_sb["bv"][0:1, ci * 512 : (ci + 1) * 512],
                                start=False,
                                stop=True,
                            )
                        dst = v_att.rearrange("p t (h e) -> p t h e", e=65)[
                            :sz, ti, ci * 8 : (ci + 1) * 8, 0:64
                        ]
                        nc.vector.tensor_copy(
                            out=dst, in_=pm[:sz].rearrange("p (h e) -> p h e", e=64)
                        )

                # q^T / k^T (feature-major)
                for wd, bnm, outT in ((wq_d, "bq", qT), (wk_d, "bk", kT)):
                    for m in range(KC):
                        w_t = p_w.tile([128, C], BF16, name="w_t")
                        nc.sync.dma_start(out=w_t, in_=wd[m])
                        for c0, csz in TCHUNKS:
                            pm = psum_mm()
                            for kc in range(KC):
                                nc.tensor.matmul(
                                    pm[:, :csz],
                                    w_t[:, kc * 128 : (kc + 1) * 128],
                                    hT[:, kc, c0 : c0 + csz],
                                    start=(kc == 0),
                                    stop=(kc == KC - 1 and bnm not in bias_sb),
                                )
                            if bnm in bias_sb:
                                nc.tensor.matmul(
                                    pm[:, :csz],
                                    bias_sb[bnm][0:1, m * 128 : (m + 1) * 128],
                                    ones_r[0:1, :csz],
                                    start=False,
                                    stop=True,
                                )
                            # alternate eviction engine to balance ACT/DVE
                            if m & 1:
                                nc.scalar.copy(
                                    out=outT[:, m, c0 : c0 + csz], in_=pm[:, :csz]
                                )
                            else:
                                nc.vector.tensor_copy(
                                    out=outT[:, m, c0 : c0 + csz], in_=pm[:, :csz]
                                )

            # ---- attention core + fused out-proj/LN2 ----
            with ExitStack() as ph1b:
                p_s = ph1b.enter_context(tc.tile_pool(name="p_s", bufs=2))
                p_xr = ph1b.enter_context(tc.tile_pool(name="p_xr", bufs=6))
                p_ln2 = ph1b.enter_context(tc.tile_pool(name="p_ln2", bufs=2))

                # deferred-by-one-head normalization multiply: when the DVE
                # reaches mult(h-1), its broadcast finished long ago, so the
                # DVE FIFO never stalls on the GpSimd hop.
                pending = []

                def flush_pending():
                    while pending:
                        dst, src_lo, r_bc_prev = pending.pop(0)
                        nc.vector.tensor_tensor(
                            out=dst, in0=dst, in1=r_bc_prev, op=ALU.mult
                        )

                for b in range(BPC):
                    sb0 = b * N
                    for h in range(H):
                        kch = h // 2
                        po = (h % 2) * 64
                        ps_oc = [
                            ps.tile([65, 512], F32, tag="o", name="ps_o", bufs=2)
                            for _ in QCHUNKS
                        ]
                        for kt in range(5):
                            koff = sb0 + kt * 128
                            ksz = min(128, N - kt * 128)
                            ps_s = ps.tile([128, N], F32, tag="s", name="ps_s", bufs=2)
                            for q0, qsz in QCHUNKS:
                                nc.tensor.matmul(
                                    ps_s[:ksz, q0 : q0 + qsz],
                                    kT[po : po + 64, kch, koff : koff + ksz],
                                    qT[po : po + 64, kch, sb0 + q0 : sb0 + q0 + qsz],
                                    start=True,
                                    stop=True,
                                )
                            e_t = p_e.tile([128, N], BF16, tag="E", name="E", bufs=3)
                            nc.scalar.activation(
                                out=e_t[:ksz], in_=ps_s[:ksz], func=AF.Exp, scale=0.125
                            )
                            for ci, (q0, qsz) in enumerate(QCHUNKS):
                                nc.tensor.matmul(
                                    ps_oc[ci][:, :qsz],
                                    v_att[:ksz, b * 5 + kt, h * 65 : (h + 1) * 65],
                                    e_t[:ksz, q0 : q0 + qsz],
                                    start=(kt == 0),
                                    stop=(kt == 4),
                                )
                        # DVE: evict unnormalized o + sums (frees psum fast)
                        s_sb = p_s.tile([1, N], F32, name="s_sb", bufs=3)
                        for ci, (q0, qsz) in enumerate(QCHUNKS):
                            nc.vector.tensor_copy(
                                out=oTb[b][po : po + 64, kch, q0 : q0 + qsz],
                                in_=ps_oc[ci][0:64, :qsz],
                            )
                            nc.vector.tensor_copy(
                                out=s_sb[:, q0 : q0 + qsz], in_=ps_oc[ci][64:65, :qsz]
                            )
                        r_sb = p_s.tile([1, N], F32, name="r_sb", bufs=3)
                        nc.vector.reciprocal_approx_fast(out=r_sb, in_=s_sb)
                        r_bc = p_s.tile([128, N], F32, name="r_bc", bufs=3)
                        nc.gpsimd.partition_broadcast(r_bc, r_sb)
                        flush_pending()
                        pending.append(
                            (
                                oTb[b][po : po + 64, kch, :],
                                None,
                                r_bc[po : po + 64],
                            )
                        )
                flush_pending()

                # attention out-proj + residual-1 + LN2 + transpose, fused
                # per token tile so PE/DVE/ACT stay busy across the boundary
                for ti, (off, sz, bt, _) in enumerate(TOK_TILES):
                    for ci in range(2):
                        pm = psum_mm()
                        for kc in range(KC):
                            nc.tensor.matmul(
                                pm[:sz],
                                oTb[bt][:, kc, off - bt * N : off - bt * N + sz],
                                wp_sb[:, kc, ci * 512 : (ci + 1) * 512],
                                start=(kc == 0),
                                stop=(kc == KC - 1 and "bp" not in bias_sb),
                            )
                        if "bp" in bias_sb:
                            nc.tensor.matmul(
                                pm[:sz],
                                ones_m[0:1, :sz],
                                bias_sb["bp"][0:1, ci * 512 : (ci + 1) * 512],
                                start=False,
                                stop=True,
                            )
                        x_t2 = p_xr.tile([128, 512], F32, name="x_t2")
                        nc.sync.dma_start(
                            out=x_t2[:sz],
                            in_=x_d[off : off + sz, ci * 512 : (ci + 1) * 512],
                        )
                        nc.vector.scalar_tensor_tensor(
                            out=out1[:sz, ti, ci * 512 : (ci + 1) * 512],
                            in0=pm[:sz],
                            scalar=1.0,
                            in1=x_t2[:sz],
                            op0=ALU.mult,
                            op1=ALU.add,
                        )
                    h_t2 = ln_tile(p_ln2, out1[:, ti, :], sz, "2")
                    transpose_into(h_t2, sz, h2T, off, tag="s")

        # ---------------- phase 2: MLP ----------------
        with ExitStack() as ph2:
            p_gu = ph2.enter_context(tc.tile_pool(name="p_gu", bufs=1))
            p_w2r = ph2.enter_context(tc.tile_pool(name="p_w2r", bufs=1))
            p_w1s = ph2.enter_context(tc.tile_pool(name="p_w1s", bufs=3))
            p_sg = ph2.enter_context(tc.tile_pool(name="p_sg", bufs=2))
            p_res = ph2.enter_context(tc.tile_pool(name="p_res", bufs=4))

            guT = p_gu.tile([128, MC, T], BF16, name="guT")
            w2sb = p_w2r.tile([128, MC, C], BF16, name="w2sb")

            # fc1 + fast-gelu (x * sigmoid(1.702 x)); W2 prefetch rides the
            # gpsimd DGE queue, one tile per fc1 step, so it hides under fc1
            # without blocking the w1 stream on the Sync queue
            for m in range(MC):
                w_t = p_w1s.tile([128, C], BF16, name="w1_t")
                nc.sync.dma_start(out=w_t, in_=w1_d[m])
                nc.gpsimd.dma_start(out=w2sb[:, m, :], in_=w2_d[m])
                for c0, csz in TCHUNKS:
                    pm = psum_mm("mm" if (m & 1) == 0 else "s")
                    for kc in range(KC):
                        nc.tensor.matmul(
                            pm[:, :csz],
                            w_t[:, kc * 128 : (kc + 1) * 128],
                            h2T[:, kc, c0 : c0 + csz],
                            start=(kc == 0),
                            stop=(kc == KC - 1 and "b1" not in bias_sb),
                        )
                    if "b1" in bias_sb:
                        nc.tensor.matmul(
                            pm[:, :csz],
                            bias_sb["b1"][0:1, m * 128 : (m + 1) * 128],
                            ones_r[0:1, :csz],
                            start=False,
                            stop=True,
                        )
                    sg = p_sg.tile([128, 512], F32, name="sg")
                    nc.scalar.activation(
                        out=sg[:, :csz], in_=pm[:, :csz], func=AF.Sigmoid, scale=1.702
                    )
                    nc.vector.tensor_tensor(
                        out=guT[:, m, c0 : c0 + csz],
                        in0=pm[:, :csz],
                        in1=sg[:, :csz],
                        op=ALU.mult,
                    )

            # fc2 + residual-2, straight to DRAM
            for ti, (off, sz, _, _) in enumerate(TOK_TILES):
                pms = [
                    psum_mm("mm"),
                    ps.tile([128, N], F32, tag="s", name="pm_s", bufs=2),
                ]
                for hc in range(MC):
                    for half in range(2):
                        nc.tensor.matmul(
                            pms[half][:sz, :512],
                            guT[:, hc, off : off + sz],
                            w2sb[:, hc, half * 512 : half * 512 + 512],
                            start=(hc == 0),
                            stop=(hc == MC - 1 and "b2" not in bias_sb),
                        )
                for half in range(2):
                    h0 = half * 512
                    if "b2" in bias_sb:
                        nc.tensor.matmul(
                            pms[half][:sz, :512],
                            ones_m[0:1, :sz],
                            bias_sb["b2"][0:1, h0 : h0 + 512],
                            start=False,
                            stop=True,
                        )
                    res = p_res.tile([128, 512], F32, name="res")
                    nc.vector.scalar_tensor_tensor(
                        out=res[:sz],
                        in0=pms[half][:sz, :512],
                        scalar=1.0,
                        in1=out1[:sz, ti, h0 : h0 + 512],
                        op0=ALU.mult,
                        op1=ALU.add,
                    )
                    nc.sync.dma_start(
                        out=out_d[off : off + sz, h0 : h0 + 512], in_=res[:sz]
                    )

    nc.compile()
    return nc


def _prepare(inputs):
    """Host-side: fold norms into weights, transpose/block, shard x."""
    f = lambda k: np.asarray(inputs[k], dtype=np.float32)
    x = f("x")
    g1, b1ln = f("norm1_g"), f("norm1_b")
    g2, b2ln = f("norm2_g"), f("norm2_b")
    Wq, Wk, Wv, Wp = f("Wq"), f("Wk"), f("Wv"), f("Wp")
    W1, W2 = f("W1"), f("W2")
    bp, b1, b2 = f("bp"), f("b1"), f("b2")

    wq = _block_lhs((Wq * g1[None, :]).T, KC)
    wk = _block_lhs((Wk * g1[None, :]).T, KC)
    wv = np.ascontiguousarray((Wv * g1[None, :]).T.reshape(KC, 128, C))
    wp = np.ascontiguousarray(Wp.T.reshape(KC, 128, C))
    w1 = _block_lhs((W1 * g2[None, :]).T, MC)
    w2 = np.ascontiguousarray(W2.T.reshape(MC, 128, C))

    biases = {
        "bq": Wq @ b1ln,
        "bk": Wk @ b1ln,
        "bv": Wv @ b1ln,
        "bp": bp,
        "b1": W1 @ b2ln + b1,
        "b2": b2,
    }
    use_bias = {k: bool(np.any(v != 0)) for k, v in biases.items()}

    weights = {
        "wq": _bf(wq),
        "wk": _bf(wk),
        "wv": _bf(wv),
        "wp": _bf(wp),
        "w1": _bf(w1),
        "w2": _bf(w2),
    }
    for k, v in biases.items():
        if use_bias[k]:
            weights[k] = _bf(v.reshape(1, -1))

    xs = x.reshape(NCORES, T, C)
    in_maps = [dict(weights, x=np.ascontiguousarray(xs[i])) for i in range(NCORES)]
    return in_maps, use_bias


_CACHE = {}


def _get_program(use_bias):
    key = tuple(sorted(use_bias.items()))
    if key not in _CACHE:
        _CACHE[key] = _build(use_bias)
    return _CACHE[key]


def run(inputs, trace=False, **kw):
    in_maps, use_bias = _prepare(inputs)
    nc = _get_program(use_bias)
    res = run_bass_kernel_spmd(
        nc, in_maps, core_ids=list(range(NCORES)), trace=trace, **kw
    )
    out = np.concatenate([res.results[i]["out"] for i in range(NCORES)], axis=0)
    return np.ascontiguousarray(out.astype(np.float32)), res


def kernel(**inputs):
    out, _ = run(inputs, trace=False)
    return out

